# revision 7
# baseline (speedup 1.0000x reference)
"""Trainium2 Bass kernel for nn_BasicNCAModel (neural cellular automaton).

Full inputs in, full outputs out. Sharding: pure data parallel over batch
(B=8 -> 8 NeuronCores, one image per core); all params replicated.

Per NCA step (per core, image x [20, 256, 256] fp32):
  perc   = concat([x, dwconv3x3_reflect(x, w_f1), dwconv3x3_reflect(x, w_f2)])
  h      = relu(W1 @ perc + b1)            # 60 -> 128, 1x1
  dx     = W2 @ h                          # 128 -> 20, 1x1
  x      = x + dx * stoch * ch_mask

Kernel formulation (all per-pixel work on the PE array):
  h_pre[o, p] = sum_{dy,dx,c} A[dx][dy*20+c, o] * x[c, py+dy, px+dx]  (reflect)
  with A[dx][g*20+c, o] = W1[o,20+c]*w1[c,dy+1,dx+1] + W1[o,40+c]*w2[c,dy+1,dx+1]
                          (+ W1[o,c] at dy=dx=0)
  This is 3 PSUM-accumulated matmuls (one per dx in {-1,0,1}) against a
  stacked SBUF tile S[61, R, W+2] holding 3 row-shifted copies of x
  (partitions 0-19: dy=0, 20-39: dy=-1, 40-59: dy=+1) plus a "penalty" row
  (partition 60, center matmul only) = -1e5 where the stochastic fire mask
  is 0, so relu(h_pre + pen) == 0 there and the update becomes exactly
  x' = x.  The channel mask is folded into W2 (rows 0-2 zeroed).
  Then: h = relu(h_pre + b1) on ScalarE (fused bias), dx = W2T @ h on PE,
  x' = x + dx on VectorE (reads the dy=0 group of S for x).

Matmuls run as float32r (full-rate on TRN2 for moving dim >= 256).
x ping-pongs through DRAM scratch tensors between steps; reflect padding is
implemented with split row DMAs and two small on-chip pad-column copies.

The stochastic mask replicates jax.random:  fold_in(key(42), step) ->
uniform(B,1,H,W) < 0.5, computed host-side on CPU jax (bit-exact vs the
reference since threefry is deterministic).
"""

import sys

if "/opt/trn_rl_repo" not in sys.path:
    sys.path.insert(0, "/opt/trn_rl_repo")

import numpy as np

C = 20
HID = 128
H = 256
W = 256
NCORES = 8
BIG = 1.0e5
FIRE_RATE = 0.5

_NC_CACHE = {}


def _build_nc(steps, n_loop=1, img_h=H, rows_per_tile=32):
    """Build + finalize the Bass module for `steps` NCA steps.

    n_loop > 1 wraps the whole steps-chain in a hardware For_i loop (each
    iteration recomputes from the input image) — used only for timing.
    """
    import concourse.bacc as bacc
    import concourse.tile as tile
    import concourse.mybir as mybir
    from contextlib import ExitStack

    f32 = mybir.dt.float32
    f32r = mybir.dt.float32r
    R = rows_per_tile
    assert img_h % R == 0 and R % 2 == 0

    nc = bacc.Bacc("TRN2", target_bir_lowering=False, debug=False,
                   num_devices=NCORES)
    x_in = nc.dram_tensor("x", [C, img_h, W], f32, kind="ExternalInput")
    bf16 = mybir.dt.bfloat16
    taps_in = nc.dram_tensor("taps", [3, 61, HID], bf16, kind="ExternalInput")
    w2t_in = nc.dram_tensor("w2t", [HID, C], bf16, kind="ExternalInput")
    b1_in = nc.dram_tensor("b1", [HID, 1], f32, kind="ExternalInput")
    pen_in = nc.dram_tensor("pen", [steps, img_h, W], bf16, kind="ExternalInput")
    y_out = nc.dram_tensor("y", [C, img_h, W], f32, kind="ExternalOutput")
    scr = [nc.dram_tensor(f"scr{i}", [C, img_h, W], f32, kind="Internal")
           for i in range(2)]

    with tile.TileContext(nc) as tc, ExitStack() as ctx:
        wpool = ctx.enter_context(tc.tile_pool(name="wpool", bufs=1))
        spool = ctx.enter_context(tc.tile_pool(name="spool", bufs=3))
        hpool = ctx.enter_context(tc.tile_pool(name="hpool", bufs=4))
        opool = ctx.enter_context(tc.tile_pool(name="opool", bufs=2))
        xcpool = ctx.enter_context(tc.tile_pool(name="xcpool", bufs=2))
        papool = ctx.enter_context(tc.tile_pool(name="papool", bufs=4, space="PSUM"))
        p2pool = ctx.enter_context(tc.tile_pool(name="p2pool", bufs=3, space="PSUM"))

        taps_sb = wpool.tile([61, 3, HID], bf16)
        nc.sync.dma_start(out=taps_sb, in_=taps_in.ap().rearrange("d k m -> k d m"))
        w2t_sb = wpool.tile([HID, C], bf16)
        nc.sync.dma_start(out=w2t_sb, in_=w2t_in[:, :])
        b1_sb = wpool.tile([HID, 1], f32)
        nc.sync.dma_start(out=b1_sb, in_=b1_in[:, :])

        def emit_step(src, dst, s):
            # Software-pipelined emission: the PE stream must not contain
            # mm2(g) right after taps(g) — it would stall waiting for the
            # ScalarE relu of the same group.  Delay mm2/add of group g until
            # after taps of group g+DELAY have been issued.
            DELAY = 3
            pend = []

            def flush_one():
                ht, p2s, xo_t, r0, r1, wb = pend.pop(0)
                p2 = p2pool.tile([C, 2, W], f32, tag="p2", name="p2")
                nc.tensor.matmul(p2, lhsT=w2t_sb[:, :], rhs=ht[:, :, :],
                                 start=True, stop=True)
                nc.vector.tensor_tensor(out=xo_t[:, r0:r1, :], in0=p2,
                                        in1=p2s,
                                        op=mybir.AluOpType.add)
                if wb is not None:
                    wb()

            for t in range(img_h // R):
                h0 = t * R
                S = spool.tile([61, R, W + 2], bf16, tag="S", name="S")
                # group 0 (partitions 0-19): dy=0 rows [h0, h0+R)
                nc.gpsimd.dma_start(out=S[0:20, :, 1:W + 1], in_=src[:, h0:h0 + R, :])
                # group 1 (partitions 20-39): dy=-1 rows, reflect at top
                if h0 == 0:
                    nc.gpsimd.dma_start(out=S[20:40, 0:1, 1:W + 1], in_=src[:, 1:2, :])
                    nc.gpsimd.dma_start(out=S[20:40, 1:R, 1:W + 1],
                                        in_=src[:, 0:R - 1, :])
                else:
                    nc.gpsimd.dma_start(out=S[20:40, :, 1:W + 1],
                                        in_=src[:, h0 - 1:h0 + R - 1, :])
                # group 2 (partitions 40-59): dy=+1 rows, reflect at bottom
                if h0 + R == img_h:
                    nc.gpsimd.dma_start(out=S[40:60, 0:R - 1, 1:W + 1],
                                        in_=src[:, h0 + 1:img_h, :])
                    nc.gpsimd.dma_start(out=S[40:60, R - 1:R, 1:W + 1],
                                        in_=src[:, img_h - 2:img_h - 1, :])
                else:
                    nc.gpsimd.dma_start(out=S[40:60, :, 1:W + 1],
                                        in_=src[:, h0 + 1:h0 + R + 1, :])
                # penalty row (partition 60), read by the center matmul only
                nc.sync.dma_start(out=S[60:61, :, 1:W + 1],
                                  in_=pen_in[s:s + 1, h0:h0 + R, :])
                # reflect pad columns: S[.,.,0] = x[..,1], S[.,.,W+1] = x[..,W-2]
                nc.gpsimd.tensor_copy(out=S[0:60, :, 0:1], in_=S[0:60, :, 2:3])
                nc.gpsimd.tensor_copy(out=S[0:60, :, W + 1:W + 2],
                                      in_=S[0:60, :, W - 1:W])

                xc = xcpool.tile([C, R, W], f32, tag="xc", name="xc")
                nc.sync.dma_start(out=xc, in_=src[:, h0:h0 + R, :])
                xo = opool.tile([C, R, W], f32, tag="xo", name="xo")
                ngroups = R // 2
                for b in range(ngroups):
                    r0, r1 = 2 * b, 2 * b + 2
                    pa = papool.tile([HID, 2, W], f32, tag="pa", name="pa")
                    nc.tensor.matmul(pa, lhsT=taps_sb[0:60, 0, :],
                                     rhs=S[0:60, r0:r1, 0:W],
                                     start=True, stop=False)
                    nc.tensor.matmul(pa, lhsT=taps_sb[0:61, 1, :],
                                     rhs=S[0:61, r0:r1, 1:W + 1],
                                     start=False, stop=False)
                    nc.tensor.matmul(pa, lhsT=taps_sb[0:60, 2, :],
                                     rhs=S[0:60, r0:r1, 2:W + 2],
                                     start=False, stop=True)
                    ht = hpool.tile([HID, 2, W], bf16, tag="ht", name="ht")
                    nc.scalar.activation(out=ht, in_=pa,
                                         func=mybir.ActivationFunctionType.Relu,
                                         bias=b1_sb[:, 0:1], scale=1.0)
                    wb = None
                    if b == ngroups - 1:
                        def wb(dst=dst, h0=h0, xo=xo):
                            nc.sync.dma_start(out=dst[:, h0:h0 + R, :], in_=xo)
                    pend.append((ht, xc[:, r0:r1, :], xo, r0, r1, wb))
                    while len(pend) > DELAY:
                        flush_one()
            while pend:
                flush_one()

        def emit_chain():
            if steps == 1:
                emit_step(x_in, y_out, 0)
                return
            emit_step(x_in, scr[0], 0)
            for s in range(1, steps - 1):
                emit_step(scr[(s - 1) % 2], scr[s % 2], s)
            emit_step(scr[(steps - 2) % 2], y_out, steps - 1)

        if n_loop == 1:
            emit_chain()
        else:
            with tc.For_i(0, n_loop):
                emit_chain()

    nc.finalize()
    return nc


def get_nc(steps, n_loop=1, img_h=H, rows_per_tile=32):
    key = (steps, n_loop, img_h, rows_per_tile)
    if key not in _NC_CACHE:
        _NC_CACHE[key] = _build_nc(steps, n_loop, img_h, rows_per_tile)
    return _NC_CACHE[key]


def _stoch_masks(steps, b, img_h=H):
    """Replicate reference RNG exactly: fold_in(key(42), step) -> uniform."""
    import jax

    cpu = jax.devices("cpu")[0]
    base_key = jax.random.key(42)
    out = np.empty((steps, b, img_h, W), dtype=bool)
    with jax.default_device(cpu):
        for s in range(steps):
            k = jax.random.fold_in(base_key, s)
            u = jax.random.uniform(k, (b, 1, img_h, W))
            out[s] = np.asarray(u[:, 0]) < FIRE_RATE
    return out


def make_host_inputs(x, w_f1, w_f2, W1, b1, W2, steps):
    """Precompute per-core DRAM inputs (taps lhsT, masked W2T, penalties)."""
    bsz = x.shape[0]
    A = np.zeros((3, 61, HID), np.float32)
    dys = (0, -1, 1)
    for j in range(3):
        for g, dy in enumerate(dys):
            c1 = w_f1[:, 0, dy + 1, j]
            c2 = w_f2[:, 0, dy + 1, j]
            A[j, 20 * g:20 * g + 20, :] = (W1[:, 20:40] * c1[None, :]).T \
                + (W1[:, 40:60] * c2[None, :]).T
            if dy == 0 and j == 1:
                A[j, 0:20, :] += W1[:, 0:20].T
    A[1, 60, :] = 1.0

    w2m = W2.copy()
    w2m[0:3, :] = 0.0
    import ml_dtypes
    w2t = np.ascontiguousarray(w2m.T).astype(ml_dtypes.bfloat16)
    b1c = np.ascontiguousarray(b1.reshape(HID, 1)).astype(np.float32)

    stoch = _stoch_masks(steps, bsz, x.shape[2])
    pen = np.where(stoch, np.float32(0.0), np.float32(-BIG)).astype(ml_dtypes.bfloat16)  # [steps,B,H,W]

    in_maps = []
    for i in range(bsz):
        in_maps.append({
            "x": np.ascontiguousarray(x[i]).astype(np.float32),
            "taps": A.astype(ml_dtypes.bfloat16),
            "w2t": w2t,
            "b1": b1c,
            "pen": np.ascontiguousarray(pen[:, i]),
        })
    return in_maps


def kernel(x, w_f1, w_f2, W1, b1, W2, steps):
    x = np.asarray(x, dtype=np.float32)
    w_f1 = np.asarray(w_f1, dtype=np.float32)
    w_f2 = np.asarray(w_f2, dtype=np.float32)
    W1 = np.asarray(W1, dtype=np.float32)
    b1 = np.asarray(b1, dtype=np.float32)
    W2 = np.asarray(W2, dtype=np.float32)
    steps = int(steps)
    if steps <= 0:
        return x.copy()

    from concourse.bass_utils import run_bass_kernel_spmd

    nc = get_nc(steps)
    in_maps = make_host_inputs(x, w_f1, w_f2, W1, b1, W2, steps)
    res = run_bass_kernel_spmd(nc, in_maps, core_ids=list(range(x.shape[0])))
    out = np.stack([res.results[i]["y"] for i in range(x.shape[0])], axis=0)
    return out.astype(np.float32)


# revision 21
# speedup vs baseline: 2.1796x; 2.1796x over previous
"""Trainium2 Bass kernel for nn_BasicNCAModel (neural cellular automaton).

Full inputs in, full outputs out. Sharding: pure data parallel over batch
(B=8 -> 8 NeuronCores, one image per core); all params replicated.

Per NCA step (per core, image x [20, 256, 256] fp32):
  perc   = concat([x, dwconv3x3_reflect(x, w_f1), dwconv3x3_reflect(x, w_f2)])
  h      = relu(W1 @ perc + b1)            # 60 -> 128, 1x1
  dx     = W2 @ h                          # 128 -> 20, 1x1
  x      = x + dx * stoch * ch_mask

Kernel formulation (all per-pixel work on the PE array):
  h_pre[o, p] = sum_{dy,dx,c} A[dx][dy*20+c, o] * x[c, py+dy, px+dx]  (reflect)
  with A[dx][g*20+c, o] = W1[o,20+c]*w1[c,dy+1,dx+1] + W1[o,40+c]*w2[c,dy+1,dx+1]
                          (+ W1[o,c] at dy=dx=0)
  This is 3 PSUM-accumulated matmuls (one per dx in {-1,0,1}) against a
  stacked SBUF tile S[61, R, W+2] holding 3 row-shifted copies of x
  (partitions 0-19: dy=0, 20-39: dy=-1, 40-59: dy=+1) plus a "penalty" row
  (partition 60, center matmul only) = -1e5 where the stochastic fire mask
  is 0, so relu(h_pre + pen) == 0 there and the update becomes exactly
  x' = x.  The channel mask is folded into W2 (rows 0-2 zeroed).
  Then: h = relu(h_pre + b1) on ScalarE (fused bias), dx = W2T @ h on PE,
  x' = x + dx on VectorE (reads the dy=0 group of S for x).

Matmuls run as float32r (full-rate on TRN2 for moving dim >= 256).
x ping-pongs through DRAM scratch tensors between steps; reflect padding is
implemented with split row DMAs and two small on-chip pad-column copies.

The stochastic mask replicates jax.random:  fold_in(key(42), step) ->
uniform(B,1,H,W) < 0.5, computed host-side on CPU jax (bit-exact vs the
reference since threefry is deterministic).
"""

import sys

if "/opt/trn_rl_repo" not in sys.path:
    sys.path.insert(0, "/opt/trn_rl_repo")

import numpy as np

C = 20
HID = 128
H = 256
W = 256
NCORES = 8
BIG = 1.0e5
FIRE_RATE = 0.5

_NC_CACHE = {}


def _build_nc(steps, n_loop=1, img_h=H, rows_per_tile=32):
    """Build + finalize the Bass module for `steps` NCA steps.

    n_loop > 1 wraps the whole steps-chain in a hardware For_i loop (each
    iteration recomputes from the input image) — used only for timing.
    """
    import concourse.bacc as bacc
    import concourse.tile as tile
    import concourse.mybir as mybir
    from contextlib import ExitStack

    f32 = mybir.dt.float32
    f32r = mybir.dt.float32r
    R = rows_per_tile
    assert img_h % R == 0 and R % 2 == 0

    nc = bacc.Bacc("TRN2", target_bir_lowering=False, debug=False,
                   num_devices=NCORES)
    x_in = nc.dram_tensor("x", [C, img_h, W], f32, kind="ExternalInput")
    tapsA_in = nc.dram_tensor("tapsA", [125, HID], f32, kind="ExternalInput")
    tapsB_in = nc.dram_tensor("tapsB", [60, HID], f32, kind="ExternalInput")
    w2t_in = nc.dram_tensor("w2t", [HID, C], f32, kind="ExternalInput")
    b1_in = nc.dram_tensor("b1", [HID, 1], f32, kind="ExternalInput")
    pen_in = nc.dram_tensor("pen", [steps, img_h, W], f32, kind="ExternalInput")
    y_out = nc.dram_tensor("y", [C, img_h, W], f32, kind="ExternalOutput")
    scr = [nc.dram_tensor(f"scr{i}", [C, img_h, W], f32, kind="Internal")
           for i in range(2)]

    with tile.TileContext(nc) as tc, ExitStack() as ctx:
        wpool = ctx.enter_context(tc.tile_pool(name="wpool", bufs=1))
        spool = ctx.enter_context(tc.tile_pool(name="spool", bufs=3))
        hpool = ctx.enter_context(tc.tile_pool(name="hpool", bufs=4))
        opool = ctx.enter_context(tc.tile_pool(name="opool", bufs=2))
        papool = ctx.enter_context(tc.tile_pool(name="papool", bufs=4, space="PSUM"))
        p2pool = ctx.enter_context(tc.tile_pool(name="p2pool", bufs=3, space="PSUM"))

        tapsA_sb = wpool.tile([125, HID], f32r)
        nc.sync.dma_start(out=tapsA_sb, in_=tapsA_in[:, :].bitcast(f32r))
        # tapsB lives at partitions 64-123 so its base matches MM_B's rhs
        tapsB_sb = wpool.tile([124, HID], f32r)
        nc.sync.dma_start(out=tapsB_sb[64:124, :], in_=tapsB_in[:, :].bitcast(f32r))
        w2t_sb = wpool.tile([HID, C], f32r)
        nc.sync.dma_start(out=w2t_sb, in_=w2t_in[:, :].bitcast(f32r))
        b1_sb = wpool.tile([HID, 1], f32)
        nc.sync.dma_start(out=b1_sb, in_=b1_in[:, :])

        def emit_step(src, dst, s):
            # Software-pipelined emission: the PE stream must not contain
            # mm2(g) right after taps(g) — it would stall waiting for the
            # ScalarE relu of the same group.  Delay mm2/add of group g until
            # after taps of group g+DELAY have been issued.
            DELAY = 3
            pend = []

            def flush_one():
                ht, p2s, xo_t, r0, r1, wb = pend.pop(0)
                p2 = p2pool.tile([C, 2, W], f32, tag="p2", name="p2")
                nc.tensor.matmul(p2, lhsT=w2t_sb[:, :], rhs=ht[:, :, :],
                                 start=True, stop=True)
                nc.vector.tensor_tensor(out=xo_t[:, r0:r1, :], in0=p2,
                                        in1=p2s.bitcast(f32),
                                        op=mybir.AluOpType.add)
                if wb is not None:
                    wb()

            for t in range(img_h // R):
                h0 = t * R
                # Stagger stack (R+1 rows each), 6 shifted x copies + penalty:
                #   p0-19    g3 (dy -1, dx 0):  S[.,r,u] = x[h0-1+r, u-1]
                #   p20-39   g4 (dy -1, dx -1): copy of g3 shifted right
                #   p40-63   g5 (dy -1, dx +1): copy of g3 shifted left (+4
                #            dummy partitions 60-63, zero weights)
                #   p64-83   g0 (dy 0, dx 0):   S[.,r,u] = x[h0+r, u-1]
                #   p84-103  g1 (dy 0, dx -1),  p104-123 g2 (dy 0, dx +1)
                #   p124     pen
                # MM_A (K=125) at rows r0:r1 covers the 6 dy<=0 taps + pen;
                # MM_B (K=60, base partition 64) reads rows r0+1:r1+1 of
                # g0-g2, giving the three dy=+1 taps.  The final residual add
                # reads exact-x from g3 rows r0+1:r1+1 (base partition 0).
                # Reflect pad columns are filled on g3/g0 (32-aligned bases)
                # BEFORE the dx-shift copies, so the shifts inherit them.
                S = spool.tile([125, R + 1, W + 2], f32r, tag="S", name="S")
                # g3 rows 0..R = x rows h0-1 .. h0+R-1 (reflect at top)
                if h0 == 0:
                    nc.sync.dma_start(out=S[0:20, 0:1, 1:W + 1],
                                      in_=src[:, 1:2, :].bitcast(f32r))
                    nc.sync.dma_start(out=S[0:20, 1:R + 1, 1:W + 1],
                                      in_=src[:, 0:R, :].bitcast(f32r))
                else:
                    nc.sync.dma_start(out=S[0:20, 0:R + 1, 1:W + 1],
                                      in_=src[:, h0 - 1:h0 + R, :].bitcast(f32r))
                # g0 rows 0..R = x rows h0 .. h0+R (reflect at bottom)
                if h0 + R == img_h:
                    nc.sync.dma_start(out=S[64:84, 0:R, 1:W + 1],
                                      in_=src[:, h0:h0 + R, :].bitcast(f32r))
                    nc.sync.dma_start(out=S[64:84, R:R + 1, 1:W + 1],
                                      in_=src[:, img_h - 2:img_h - 1, :].bitcast(f32r))
                else:
                    nc.sync.dma_start(out=S[64:84, 0:R + 1, 1:W + 1],
                                      in_=src[:, h0:h0 + R + 1, :].bitcast(f32r))
                # reflect pad columns on the source groups (32-aligned bases)
                nc.gpsimd.tensor_copy(out=S[0:20, :, 0:1], in_=S[0:20, :, 2:3])
                nc.gpsimd.tensor_copy(out=S[0:20, :, W + 1:W + 2],
                                      in_=S[0:20, :, W - 1:W])
                nc.gpsimd.tensor_copy(out=S[64:84, :, 0:1], in_=S[64:84, :, 2:3])
                nc.gpsimd.tensor_copy(out=S[64:84, :, W + 1:W + 2],
                                      in_=S[64:84, :, W - 1:W])
                # dx-shifted duplicates (on-fabric SBUF->SBUF, pads included)
                nc.sync.dma_start(out=S[20:40, :, 1:W + 2], in_=S[0:20, :, 0:W + 1])
                nc.sync.dma_start(out=S[40:64, :, 0:W + 1], in_=S[0:24, :, 1:W + 2])
                nc.sync.dma_start(out=S[84:104, :, 1:W + 2], in_=S[64:84, :, 0:W + 1])
                nc.sync.dma_start(out=S[104:124, :, 0:W + 1], in_=S[64:84, :, 1:W + 2])
                # penalty row
                nc.sync.dma_start(out=S[124:125, 0:R, 1:W + 1],
                                  in_=pen_in[s:s + 1, h0:h0 + R, :].bitcast(f32r))

                xo = opool.tile([C, R, W], f32, tag="xo", name="xo")
                ngroups = R // 2
                for b in range(ngroups):
                    r0, r1 = 2 * b, 2 * b + 2
                    pa = papool.tile([HID, 2, W], f32, tag="pa", name="pa")
                    nc.tensor.matmul(pa, lhsT=tapsA_sb[:, :],
                                     rhs=S[0:125, r0:r1, 1:W + 1],
                                     start=True, stop=False)
                    nc.tensor.matmul(pa, lhsT=tapsB_sb[64:124, :],
                                     rhs=S[64:124, r0 + 1:r1 + 1, 1:W + 1],
                                     start=False, stop=True)
                    ht = hpool.tile([HID, 2, W], f32r, tag="ht", name="ht")
                    nc.scalar.activation(out=ht, in_=pa,
                                         func=mybir.ActivationFunctionType.Relu,
                                         bias=b1_sb[:, 0:1], scale=1.0)
                    wb = None
                    if b == ngroups - 1:
                        def wb(dst=dst, h0=h0, xo=xo):
                            nc.sync.dma_start(out=dst[:, h0:h0 + R, :], in_=xo)
                    pend.append((ht, S[0:20, r0 + 1:r1 + 1, 1:W + 1], xo, r0, r1, wb))
                    while len(pend) > DELAY:
                        flush_one()
            while pend:
                flush_one()

        def emit_chain():
            if steps == 1:
                emit_step(x_in, y_out, 0)
                return
            emit_step(x_in, scr[0], 0)
            for s in range(1, steps - 1):
                emit_step(scr[(s - 1) % 2], scr[s % 2], s)
            emit_step(scr[(steps - 2) % 2], y_out, steps - 1)

        if n_loop == 1:
            emit_chain()
        else:
            with tc.For_i(0, n_loop):
                emit_chain()

    nc.finalize()
    return nc


def get_nc(steps, n_loop=1, img_h=H, rows_per_tile=32):
    key = (steps, n_loop, img_h, rows_per_tile)
    if key not in _NC_CACHE:
        _NC_CACHE[key] = _build_nc(steps, n_loop, img_h, rows_per_tile)
    return _NC_CACHE[key]


def _stoch_masks(steps, b, img_h=H):
    """Replicate reference RNG exactly: fold_in(key(42), step) -> uniform."""
    import jax

    cpu = jax.devices("cpu")[0]
    base_key = jax.random.key(42)
    out = np.empty((steps, b, img_h, W), dtype=bool)
    with jax.default_device(cpu):
        for s in range(steps):
            k = jax.random.fold_in(base_key, s)
            u = jax.random.uniform(k, (b, 1, img_h, W))
            out[s] = np.asarray(u[:, 0]) < FIRE_RATE
    return out


def tap_matrices(w_f1, w_f2, W1):
    def coef(dy, dx):
        c1 = w_f1[:, 0, dy + 1, dx + 1]
        c2 = w_f2[:, 0, dy + 1, dx + 1]
        return ((W1[:, 20:40] * c1[None, :]).T
                + (W1[:, 40:60] * c2[None, :]).T).astype(np.float32)

    # MM_A groups (kernel partition layout; 60-63 are dummy partitions):
    A1 = np.zeros((125, HID), np.float32)
    A1[0:20] = coef(-1, 0)
    A1[20:40] = coef(-1, -1)
    A1[40:60] = coef(-1, 1)
    A1[64:84] = coef(0, 0) + W1[:, 0:20].T
    A1[84:104] = coef(0, -1)
    A1[104:124] = coef(0, 1)
    A1[124, :] = 1.0
    # MM_B groups (read one row down): (+1,0), (+1,-1), (+1,+1)
    A2 = np.concatenate([coef(1, 0), coef(1, -1), coef(1, 1)], axis=0)
    return A1, A2


def make_host_inputs(x, w_f1, w_f2, W1, b1, W2, steps):
    """Precompute per-core DRAM inputs (taps lhsT, masked W2T, penalties)."""
    bsz = x.shape[0]
    A1, A2 = tap_matrices(w_f1, w_f2, W1)

    w2m = W2.copy()
    w2m[0:3, :] = 0.0
    w2t = np.ascontiguousarray(w2m.T).astype(np.float32)
    b1c = np.ascontiguousarray(b1.reshape(HID, 1)).astype(np.float32)

    stoch = _stoch_masks(steps, bsz, x.shape[2])
    pen = np.where(stoch, np.float32(0.0), np.float32(-BIG))  # [steps,B,H,W]

    in_maps = []
    for i in range(bsz):
        in_maps.append({
            "x": np.ascontiguousarray(x[i]).astype(np.float32),
            "tapsA": A1,
            "tapsB": A2,
            "w2t": w2t,
            "b1": b1c,
            "pen": np.ascontiguousarray(pen[:, i]),
        })
    return in_maps


def kernel(x, w_f1, w_f2, W1, b1, W2, steps):
    x = np.asarray(x, dtype=np.float32)
    w_f1 = np.asarray(w_f1, dtype=np.float32)
    w_f2 = np.asarray(w_f2, dtype=np.float32)
    W1 = np.asarray(W1, dtype=np.float32)
    b1 = np.asarray(b1, dtype=np.float32)
    W2 = np.asarray(W2, dtype=np.float32)
    steps = int(steps)
    if steps <= 0:
        return x.copy()

    from concourse.bass_utils import run_bass_kernel_spmd

    nc = get_nc(steps)
    in_maps = make_host_inputs(x, w_f1, w_f2, W1, b1, W2, steps)
    res = run_bass_kernel_spmd(nc, in_maps, core_ids=list(range(x.shape[0])))
    out = np.stack([res.results[i]["y"] for i in range(x.shape[0])], axis=0)
    return out.astype(np.float32)
